# revision 1
# baseline (speedup 1.0000x reference)
"""Trainium2 Bass kernel for nn_Attention_67336497266780.

Single-head attention, B=8 S=2048 E=1024 H=64, data-parallel over batch:
each of the 8 NeuronCores computes one batch element end to end.

Per-core algorithm (scores-transposed formulation -- no attention-matrix
transpose is ever needed):
  1. Xq -> PE-transpose (128x128 blocks) -> XqT; QT[64,2048] = Wq^T XqT + bq.
     Same for Xk -> KT and Xv -> VT (f32r / tf32 matmuls, 1 cyc/row).
  2. V'[128k, 16, 65] bf16 = [transpose(VT) | ones]  (extra ones column makes
     the PV matmul accumulate the softmax denominator for free).
  3. Per k-tile m (16): scoresT_m[128k, 2048q] = KT-slice^T @ QT (f32r);
     expT_m = exp(0.125 * scoresT_m) on ScalarE (psum -> sbuf, bf16);
     outT[65, 2048q] += V'_m^T @ expT_m (bf16, accumulated in PSUM f32).
  4. Epilogue: transpose outT back to [2048, 65]; divide by column 64
     (the denominator) via DVE reciprocal + per-partition scale; DMA out.

self-contained: hardcodes shapes; builds and compiles the graph once per
process and caches the jitted PJRT executable for subsequent calls.
"""
import sys

sys.path.insert(0, "/opt/trn_rl_repo")

from contextlib import ExitStack

import numpy as np

B = 8
P = 128
S = 2048
E = 1024
H = 64
EC = E // P
ST = S // P
NPAIR = ST // 2
QC = 4
QCHUNK = S // QC

_CACHE = {}


def _build_graph():
    import concourse.mybir as mybir
    import concourse.tile as tile
    from concourse import bacc
    from concourse.masks import make_identity

    F32 = mybir.dt.float32
    F32R = mybir.dt.float32r
    BF16 = mybir.dt.bfloat16
    proj_dt, scores_dt, transpose_dt, pv_dt = F32R, F32R, F32, BF16

    nc = bacc.Bacc("TRN2", target_bir_lowering=False, debug=False)

    xq_ext = nc.dram_tensor("query", [S, E], F32, kind="ExternalInput")
    xk_ext = nc.dram_tensor("key", [S, E], F32, kind="ExternalInput")
    xv_ext = nc.dram_tensor("value", [S, E], F32, kind="ExternalInput")
    wq_ext = nc.dram_tensor("Wq", [E, H], F32, kind="ExternalInput")
    wk_ext = nc.dram_tensor("Wk", [E, H], F32, kind="ExternalInput")
    wv_ext = nc.dram_tensor("Wv", [E, H], F32, kind="ExternalInput")
    bq_ext = nc.dram_tensor("bq", [H], F32, kind="ExternalInput")
    bk_ext = nc.dram_tensor("bk", [H], F32, kind="ExternalInput")
    bv_ext = nc.dram_tensor("bv", [H], F32, kind="ExternalInput")
    out_ext = nc.dram_tensor("out", [S, H], F32, kind="ExternalOutput")

    ctx = ExitStack()
    with tile.TileContext(nc) as tc, ctx:
        const = ctx.enter_context(tc.tile_pool(name="const", bufs=1))
        persist = ctx.enter_context(tc.tile_pool(name="persist", bufs=1))
        xpool = ctx.enter_context(tc.tile_pool(name="xpool", bufs=3))
        xtpool = ctx.enter_context(tc.tile_pool(name="xtpool", bufs=2))
        exppool = ctx.enter_context(tc.tile_pool(name="exppool", bufs=3))
        ps_work = ctx.enter_context(tc.tile_pool(name="ps_work", bufs=4, space="PSUM"))
        ps_out = ctx.enter_context(tc.tile_pool(name="ps_out", bufs=1, space="PSUM"))

        ident = const.tile([P, P], transpose_dt)
        make_identity(nc, ident)

        w_sb, b_sb = {}, {}
        for name, wext, bext in (("q", wq_ext, bq_ext), ("k", wk_ext, bk_ext),
                                 ("v", wv_ext, bv_ext)):
            w_raw = const.tile([P, EC, H], F32, name=f"wraw{name}")
            nc.sync.dma_start(w_raw[:], wext.rearrange("(o p) h -> p o h", p=P))
            w = const.tile([P, EC, H], proj_dt, name=f"w{name}")
            nc.scalar.copy(out=w[:], in_=w_raw[:])
            w_sb[name] = w
            b = const.tile([H, 1], F32, name=f"b{name}")
            nc.sync.dma_start(b[:], bext[:].unsqueeze(1))
            b_sb[name] = b

        qt_sb = persist.tile([H, S], scores_dt, name="qt")
        kt_sb = persist.tile([H, S], scores_dt, name="kt")
        vt_sb = persist.tile([H, S], F32, name="vt")
        vp_sb = persist.tile([P, ST, H + 1], pv_dt, name="vprime")
        nc.vector.memset(vp_sb[:, :, H:H + 1], 1.0)

        def copy_op(i, out, in_):
            if i % 2 == 0:
                nc.scalar.copy(out=out, in_=in_)
            else:
                nc.vector.tensor_copy(out=out, in_=in_)

        def sweep_pair(xext, dst_sb, bias, pair, tag):
            x_t = xpool.tile([P, 2, E], F32, tag="x")
            s0 = pair * 2 * P
            nc.sync.dma_start(x_t[:, 0], xext[s0:s0 + P, :])
            nc.sync.dma_start(x_t[:, 1], xext[s0 + P:s0 + 2 * P, :])
            xt_t = xtpool.tile([P, EC, 2 * P], proj_dt, tag="xt")
            for c in range(EC):
                t_ps = ps_work.tile([P, 2 * P], transpose_dt, tag="ps")
                for j in range(2):
                    nc.tensor.transpose(
                        t_ps[:, j * P:(j + 1) * P],
                        x_t[:, j, c * P:(c + 1) * P].bitcast(transpose_dt),
                        ident)
                copy_op(c, xt_t[:, c], t_ps[:])
            proj_ps = ps_work.tile([H, 2 * P], F32, tag="ps")
            w = w_sb[tag]
            for c in range(EC):
                nc.tensor.matmul(
                    proj_ps[:], lhsT=w[:, c], rhs=xt_t[:, c],
                    start=(c == 0), stop=(c == EC - 1))
            sl = slice(pair * 2 * P, (pair + 1) * 2 * P)
            nc.vector.tensor_scalar(
                out=dst_sb[:, sl], in0=proj_ps[:], scalar1=bias,
                scalar2=None, op0=mybir.AluOpType.add)

        def build_vprime(m):
            t_ps = ps_work.tile([P, H], F32, tag="ps")
            nc.tensor.transpose(
                t_ps[:], vt_sb[:, m * P:(m + 1) * P],
                ident.bitcast(F32)[:H, :H])
            copy_op(m, vp_sb[:, m, 0:H], t_ps[:])

        outT_ps = ps_out.tile([H + 1, S], F32, name="outT")

        def attend_ktile(m):
            kt_slice = kt_sb[:, m * P:(m + 1) * P]
            exp_t = exppool.tile([P, S], pv_dt, tag="exp")
            for qc in range(QC):
                sc_ps = ps_work.tile([P, QCHUNK], F32, tag="ps")
                nc.tensor.matmul(
                    sc_ps[:], lhsT=kt_slice,
                    rhs=qt_sb[:, qc * QCHUNK:(qc + 1) * QCHUNK],
                    start=True, stop=True)
                nc.scalar.activation(
                    exp_t[:, qc * QCHUNK:(qc + 1) * QCHUNK], sc_ps[:],
                    mybir.ActivationFunctionType.Exp, scale=0.125)
            for qc in range(QC):
                nc.tensor.matmul(
                    outT_ps[:, qc * QCHUNK:(qc + 1) * QCHUNK],
                    lhsT=vp_sb[:, m],
                    rhs=exp_t[:, qc * QCHUNK:(qc + 1) * QCHUNK],
                    start=(m == 0), stop=(m == ST - 1),
                    skip_group_check=True)

        for pair in range(NPAIR):
            sweep_pair(xq_ext, qt_sb, b_sb["q"], pair, "q")
        for pair in range(NPAIR):
            sweep_pair(xk_ext, kt_sb, b_sb["k"], pair, "k")
            sweep_pair(xv_ext, vt_sb, b_sb["v"], pair, "v")
            for m in (2 * pair, 2 * pair + 1):
                build_vprime(m)
                attend_ktile(m)

        outT_sb = persist.tile([H + 1, S], F32, name="outT_sb")
        for qc in range(QC):
            copy_op(qc, outT_sb[:, qc * QCHUNK:(qc + 1) * QCHUNK],
                    outT_ps[:, qc * QCHUNK:(qc + 1) * QCHUNK])
        out_sb = persist.tile([P, ST, H], F32, name="out_sb")
        rc_sb = persist.tile([P, ST], F32, name="rc")
        for m in range(ST):
            o_ps = ps_work.tile([P, H + 1], F32, tag="ps")
            nc.tensor.transpose(
                o_ps[:], outT_sb[:, m * P:(m + 1) * P],
                ident.bitcast(F32)[:H + 1, :H + 1])
            nc.vector.reciprocal(rc_sb[:, m:m + 1], o_ps[:, H:H + 1])
            nc.vector.tensor_scalar(
                out=out_sb[:, m], in0=o_ps[:, 0:H], scalar1=rc_sb[:, m:m + 1],
                scalar2=None, op0=mybir.AluOpType.mult)
            nc.sync.dma_start(out_ext[m * P:(m + 1) * P, :], out_sb[:, m])

    nc.compile()
    return nc


def _get_runner():
    if "runner" in _CACHE:
        return _CACHE["runner"]

    import functools
    import traceback

    import jax
    from jax.experimental.shard_map import shard_map
    from jax.sharding import Mesh, NamedSharding, PartitionSpec

    import concourse.mybir as mybir
    from concourse import bass2jax
    from concourse.bass2jax import _bass_exec_p, partition_id_tensor

    bass2jax.install_neuronx_cc_hook()
    import libneuronxla
    hook = libneuronxla.neuronx_cc
    if not getattr(hook, "_verbose_wrapped", False):
        @functools.wraps(hook)
        def wrapped(*a, **k):
            try:
                return hook(*a, **k)
            except BaseException:
                traceback.print_exc()
                sys.stderr.flush()
                raise
        wrapped._verbose_wrapped = True
        libneuronxla.neuronx_cc = wrapped

    nc = _build_graph()

    partition_name = nc.partition_id_tensor.name if nc.partition_id_tensor else None
    in_names, out_names, out_avals, zero_outs = [], [], [], []
    for alloc in nc.m.functions[0].allocations:
        if not isinstance(alloc, mybir.MemoryLocationSet):
            continue
        name = alloc.memorylocations[0].name
        if alloc.kind == "ExternalInput":
            if name != partition_name:
                in_names.append(name)
        elif alloc.kind == "ExternalOutput":
            out_names.append(name)
            shape = tuple(alloc.tensor_shape)
            dtype = mybir.dt.np(alloc.dtype)
            out_avals.append(jax.core.ShapedArray(shape, dtype))
            zero_outs.append(np.zeros(shape, dtype))
    n_params = len(in_names)
    n_outs = len(out_avals)
    all_in_names = list(in_names) + out_names
    if partition_name is not None:
        all_in_names.append(partition_name)
    donate = tuple(range(n_params, n_params + n_outs))

    def _body(*args):
        operands = list(args)
        if partition_name is not None:
            operands.append(partition_id_tensor())
        outs = _bass_exec_p.bind(
            *operands,
            out_avals=tuple(out_avals),
            in_names=tuple(all_in_names),
            out_names=tuple(out_names),
            lowering_input_output_aliases=(),
            sim_require_finite=True,
            sim_require_nnan=True,
            nc=nc,
        )
        return tuple(outs)

    devices = jax.devices()[:B]
    mesh = Mesh(np.asarray(devices), ("core",))
    in_specs = (PartitionSpec("core"),) * (n_params + n_outs)
    out_specs = (PartitionSpec("core"),) * len(out_names)
    sharded = jax.jit(
        shard_map(_body, mesh=mesh, in_specs=in_specs,
                  out_specs=out_specs, check_rep=False),
        donate_argnums=donate, keep_unused=True)

    runner = {
        "sharded": sharded, "in_names": in_names, "out_names": out_names,
        "out_avals": out_avals, "zero_outs": zero_outs, "mesh": mesh,
    }
    _CACHE["runner"] = runner
    return runner


def kernel(**inputs):
    r = _get_runner()
    per_core = {
        "query": inputs["query"], "key": inputs["key"], "value": inputs["value"],
    }
    shared = {k: np.asarray(inputs[k]) for k in
              ("Wq", "Wk", "Wv", "bq", "bk", "bv")}

    concat_in = []
    for name in r["in_names"]:
        if name in per_core:
            arr = np.asarray(per_core[name], dtype=np.float32)
            concat_in.append(np.concatenate([arr[b] for b in range(B)], axis=0))
        else:
            arr = shared[name].astype(np.float32, copy=False)
            concat_in.append(np.concatenate([arr] * B, axis=0))
    concat_zeros = [
        np.zeros((B * z.shape[0], *z.shape[1:]), z.dtype) for z in r["zero_outs"]
    ]
    out_arrs = r["sharded"](*concat_in, *concat_zeros)
    (out_name,) = r["out_names"]
    (aval,) = r["out_avals"]
    out = np.asarray(out_arrs[0]).reshape(B, *aval.shape)
    return out.astype(np.float32, copy=False)


if __name__ == "__main__":
    rng = np.random.default_rng(0)
    fake = {
        "query": rng.standard_normal((B, S, E), dtype=np.float32),
        "key": rng.standard_normal((B, S, E), dtype=np.float32),
        "value": rng.standard_normal((B, S, E), dtype=np.float32),
        "Wq": rng.standard_normal((E, H), dtype=np.float32) / 32,
        "bq": np.zeros(H, np.float32),
        "Wk": rng.standard_normal((E, H), dtype=np.float32) / 32,
        "bk": np.zeros(H, np.float32),
        "Wv": rng.standard_normal((E, H), dtype=np.float32) / 32,
        "bv": np.zeros(H, np.float32),
    }
    out = kernel(**fake)
    print("kernel out:", out.shape, out.dtype)
